# revision 17
# baseline (speedup 1.0000x reference)
"""Causal multi-head self-attention on 8 Trainium2 NeuronCores.

Sharding: head-parallel. Each of the 8 cores owns 2 of the 16 heads:
it computes Q/K/V for its heads (full sequence), runs causal flash
attention for them entirely on-chip, applies its slice of the output
projection, and writes a full-shape partial output. The host sums the
8 partials.

v2 schedule: the kernel is jointly PE- and ACT(exp)-bound, so the
emission order interleaves at kb-block granularity: QKV-projection and
output-projection matmuls are queued as "fillers" and dropped one or
two at a time between the score/AV matmuls of the attention inner
loop. The PE never idles (stays at max p-state) while the Scalar
engine streams exp calls; output projection runs inline per q-tile so
its DMA overlaps the whole kernel instead of forming a tail.

Layout:
  - x is cast to bf16 on host and staged transposed; one DMA per
    (batch, 512-token) tile loads all 8 d-blocks.
  - Q^T, K^T are (128 = [h0|h1] x 64) x t, the exact lhsT/rhs layout
    the transposed score matmuls need; score pairs dual-issue on the
    PE via row-disjoint tile_position quadrants.
  - exp runs on ScalarE straight out of PSUM (scale=1/8 fused), a
    single call per k-block covering both heads (3D AP on diagonals).
  - V is stored per (batch, kblock) as 129 columns [v_h0 | ones |
    v_h1]; the shared ones column makes both heads' AV matmuls emit
    the softmax denominator as an extra output row for free.
  - Causal masking: diagonal blocks are narrowed to the valid q range
    and the 128-column boundary gets a precomputed 0/1 triangle
    multiply after exp.
  - Normalization at AV eviction: reciprocal row broadcast over
    partitions (GpSimd) then one fused multiply PSUM->SBUF into ctx^T.
  - Output projection consumes ctx^T blocks as stationary operands so
    results land (t x e); one DMA per (batch, q-tile) writes them out.
"""

import numpy as np
import sys

for _p in ("/opt/trn_rl_repo", "/root/.axon_site/_ro/trn_rl_repo"):
    if _p not in sys.path:
        sys.path.append(_p)

import ml_dtypes

B = 2
S = 4096
D = 1024
H = 16
DH = 64
N_CORES = 8
HEADS_PER_CORE = H // N_CORES  # 2

_cache = {}


def _build(nc, b, s):
    import concourse.bass as bass
    import concourse.mybir as mybir
    from concourse.tile import TileContext
    from contextlib import ExitStack

    dt = mybir.dt
    AF = mybir.ActivationFunctionType
    ALU = mybir.AluOpType

    t_total = b * s          # 8192
    TT = 512                 # t tile (QKV free dim)
    n_dblk = D // 128        # 8
    QT = 512                 # q tile
    n_qt = s // QT           # per batch (8)
    KB = 128                 # k block
    n_kblk = s // KB         # 32
    scale = 1.0 / np.sqrt(DH)

    x_d = nc.dram_tensor("xT", [D, t_total], dt.bfloat16, kind="ExternalInput")
    wqkv_d = nc.dram_tensor("wqkvT", [n_dblk, 128, 3 * 128], dt.bfloat16,
                            kind="ExternalInput")
    wout_d = nc.dram_tensor("woutT", [128, D], dt.bfloat16, kind="ExternalInput")
    out_d = nc.dram_tensor("partial_out", [t_total, D], dt.bfloat16,
                           kind="ExternalOutput")

    with TileContext(nc) as tc, ExitStack() as ctx:
        const = ctx.enter_context(tc.tile_pool(name="const", bufs=1))
        wqkvT = const.tile([128, n_dblk, 3 * 128], dt.bfloat16, tag="wqkv")
        woutT = const.tile([128, D], dt.bfloat16, tag="wout")
        qT = const.tile([128, t_total], dt.bfloat16, tag="qT")
        kT = const.tile([128, t_total], dt.bfloat16, tag="kT")
        # V: per (batch, kblock) 130 cols [v_h0 | ones | v_h1 | ones]
        vst = const.tile([128, b, n_kblk, 2 * DH + 2], dt.bfloat16, tag="vst")
        ctxT = const.tile([128, t_total], dt.bfloat16, tag="ctxT")
        tri = const.tile([128, 128], dt.bfloat16, tag="tri")
        ident = const.tile([128, 128], dt.bfloat16, tag="ident")

        # split the weight load so the first QKV matmuls (which need only
        # the q|k columns) start as early as possible; wout is needed last.
        wq_r = wqkv_d.rearrange("k p e -> p k e")
        nc.sync.dma_start(wqkvT[:, :, 0:256], wq_r[:, :, 0:256])

        # ones column of vst, the 0/1 lower-triangle mask (keep k<=q: in
        # (k=partition r, q=col c) space keep c >= r), and the identity
        # for the PE transpose of V.
        nc.vector.memset(vst[:, :, :, DH], 1.0)
        nc.vector.memset(vst[:, :, :, 2 * DH + 1], 1.0)
        nc.gpsimd.memset(tri[:], 1.0)
        nc.gpsimd.affine_select(
            tri[:], tri[:], pattern=[[1, 128]], compare_op=ALU.is_ge,
            fill=0.0, base=0, channel_multiplier=-1,
        )
        nc.gpsimd.affine_select(
            ident[:], tri[:], pattern=[[1, 128]], compare_op=ALU.is_equal,
            fill=0.0, base=0, channel_multiplier=-1,
        )

        # SBUF pools
        xt_pool = ctx.enter_context(tc.tile_pool(name="xt", bufs=4))
        pt_pool = ctx.enter_context(tc.tile_pool(name="pt", bufs=6))
        vt_pool = ctx.enter_context(tc.tile_pool(name="vt", bufs=2))
        ev_pool = ctx.enter_context(tc.tile_pool(name="ev", bufs=4))
        out_sb_pool = ctx.enter_context(tc.tile_pool(name="out_sb", bufs=3))
        # PSUM: 8 banks = scores 2x2 + o65 2 + skip(qkv/transpose/outproj) 2
        sc_ps = ctx.enter_context(tc.tile_pool(name="sc_ps", bufs=2, space="PSUM"))
        o65_ps = ctx.enter_context(tc.tile_pool(name="o65_ps", bufs=1, space="PSUM"))
        skip_ps = ctx.enter_context(tc.tile_pool(name="skip_ps", bufs=2,
                                                 space="PSUM"))

        xt_tiles = {}

        def xt_load(bb, tt, split=False):
            """One DMA: all 8 d-blocks of a (bb, tt) token tile. split=True
            issues two half DMAs so the first QKV matmuls start sooner."""
            xt = xt_pool.tile([128, n_dblk, TT], dt.bfloat16, tag="xt")
            t0 = bb * s + tt * TT
            xsrc = x_d.rearrange("(k p) t -> p k t", p=128)[:, :, t0:t0 + TT]
            if split:
                nc.sync.dma_start(xt[:, 0:4, :], xsrc[:, 0:4, :])
                nc.sync.dma_start(xt[:, 4:8, :], xsrc[:, 4:8, :])
            else:
                nc.sync.dma_start(xt[:], xsrc)
            xt_tiles[(bb, tt)] = xt

        def qkv_fillers(bb, tt):
            """Emit QKV projection for (bb, tt) as a list of PE closures.

            q and k accumulate into the two skip-pool slots; v reuses
            q's slot after eviction, then 4 PE transposes scatter V into
            vst via two DVE copies each.
            """
            t0 = bb * s + tt * TT
            xt = xt_tiles.pop((bb, tt))
            state = {}

            def mk_qk(dd):
                def emit():
                    if dd == 0:
                        state["ps_q"] = skip_ps.tile([128, TT], dt.float32,
                                                     name="ps_q", tag="skip")
                        state["ps_k"] = skip_ps.tile([128, TT], dt.float32,
                                                     name="ps_k", tag="skip")
                    st = dict(start=(dd == 0), stop=(dd == n_dblk - 1))
                    nc.tensor.matmul(state["ps_q"][:], wqkvT[:, dd, 0:128],
                                     xt[:, dd, :], **st)
                    nc.tensor.matmul(state["ps_k"][:], wqkvT[:, dd, 128:256],
                                     xt[:, dd, :], **st)
                    if dd == n_dblk - 1:
                        nc.vector.tensor_copy(qT[:, t0:t0 + TT], state["ps_q"][:])
                        nc.vector.tensor_copy(kT[:, t0:t0 + TT], state["ps_k"][:])
                return emit

            def mk_v(dd):
                def emit():
                    if dd == 0:
                        state["ps_v"] = skip_ps.tile([128, TT], dt.float32,
                                                     name="ps_v", tag="skip")
                    st = dict(start=(dd == 0), stop=(dd == n_dblk - 1))
                    nc.tensor.matmul(state["ps_v"][:], wqkvT[:, dd, 256:384],
                                     xt[:, dd, :], **st)
                    if dd == n_dblk - 1:
                        vt = vt_pool.tile([128, TT], dt.bfloat16, tag="vt")
                        nc.vector.tensor_copy(vt[:], state["ps_v"][:])
                        state["vt"] = vt
                return emit

            def mk_tr(j):
                def emit():
                    ps_tv = skip_ps.tile([128, 128], dt.bfloat16, name="ps_tv",
                                         tag="skip")
                    nc.tensor.transpose(ps_tv[:],
                                        state["vt"][:, j * 128:(j + 1) * 128],
                                        ident[:])
                    kb = (tt * TT) // KB + j
                    nc.vector.tensor_copy(vst[:, bb, kb, 0:DH], ps_tv[:, 0:DH])
                    nc.vector.tensor_copy(vst[:, bb, kb, DH + 1:2 * DH + 1],
                                          ps_tv[:, DH:2 * DH])
                    # (col DH and col 2*DH+1 hold the ones columns)
                return emit

            return ([mk_qk(dd) for dd in range(n_dblk)]
                    + [mk_v(dd) for dd in range(n_dblk)]
                    + [mk_tr(j) for j in range(TT // 128)])

        def outproj_fillers(bb, qt):
            """Output projection for (bb, qt): 8 matmul closures + 1 DMA."""
            tq0 = bb * s + qt * QT
            state = {}

            def mk(i):
                tb, e = divmod(i, 2)

                def emit():
                    if i == 0:
                        state["ob"] = out_sb_pool.tile(
                            [128, QT // 128, D], dt.bfloat16, name="ob",
                            tag="ob")
                    ps = skip_ps.tile([128, 512], dt.float32, name="ps_op",
                                      tag="skip")
                    t0 = tq0 + tb * 128
                    nc.tensor.matmul(ps[:], ctxT[:, t0:t0 + 128],
                                     woutT[:, e * 512:(e + 1) * 512])
                    nc.vector.tensor_copy(
                        state["ob"][:, tb, e * 512:(e + 1) * 512], ps[:])
                    if i == 7:
                        nc.sync.dma_start(
                            out_d.rearrange("(u j p) e -> p u j e", p=128,
                                            j=QT // 128)[:, bb * n_qt + qt],
                            state["ob"][:])
                return emit

            return [mk(i) for i in range(8)]

        urgent_q = []
        lazy_q = []
        lazy_st = {"credit": 0.0, "ingroup": 0}

        def pop_fillers(n_urgent, n_lazy):
            for _ in range(min(n_urgent, len(urgent_q))):
                urgent_q.pop(0)()
            for _ in range(min(n_lazy, len(lazy_q))):
                lazy_q.pop(0)()

        def pop_lazy_slot(rate, allow):
            """Credit-paced, group-atomic lazy pops. Once an 8-item outproj
            group starts, it finishes at 2/slot (credit goes negative and
            pauses later pops) so its final DMA issues promptly; `allow`
            gates pops off the first slots of an attention so a freshly
            queued group never head-blocks the PE on the previous tile's
            still-running normalize chain."""
            lazy_st["credit"] += rate
            avail = len(lazy_q)
            if not allow or avail == 0:
                return
            if lazy_st["ingroup"] > 0:
                n = min(2, lazy_st["ingroup"], avail)
            elif lazy_st["credit"] >= 1.0:
                n = min(2, avail)
                lazy_st["ingroup"] = 8
            else:
                return
            for _ in range(n):
                lazy_q.pop(0)()
            lazy_st["credit"] -= n
            lazy_st["ingroup"] -= n

        def attention(bb, qt):
            """One q-tile of causal attention for both heads of batch bb.

            Emits scores(kb+1) before AV(kb) so the PE is never
            head-of-line blocked on exp(kb); fillers are popped between
            iterations to absorb the remaining ACT/PE rate mismatch.
            """
            tq0 = bb * s + qt * QT
            o65_h0 = o65_ps.tile([DH + 1, QT], dt.float32, tag="o65h0")
            o65_h1 = o65_ps.tile([DH + 1, QT], dt.float32, tag="o65h1")
            nkb = (qt + 1) * QT // KB
            # urgent fillers (next tile's QKV) must drain within this
            # attention; lazy fillers (outproj) pace uniformly over the
            # remaining kb slots of the whole kernel so the late q-tiles
            # (which have no QKV work left) still get PE filler.
            per_slot_u = max(1, -(-len(urgent_q) // nkb))
            rem_slots = self_rem_slots[0]
            last = (bb == b - 1 and qt == n_qt - 1)
            # uniform credit pacing: the outproj backlog drains evenly over
            # ALL remaining kb slots of the kernel, so the late q-tiles
            # (which have no QKV filler) keep PE work; the last tile holds
            # back a reserve for its normalize/outproj tail.
            reserve = 8 if last else 0
            rate = max(len(lazy_q) - reserve, 0) / max(rem_slots, 1)

            pts = {}

            def scores_exp(kb):
                tk0 = bb * s + kb * KB
                j = kb - qt * (QT // KB)  # >= 0 on the diagonal
                qc0 = max(j, 0) * KB      # first valid local q column
                w = QT - qc0
                ps_s = sc_ps.tile([128, 2, QT], dt.float32, tag="ps_s")
                nc.tensor.matmul(ps_s[:, 0, 0:w], kT[0:64, tk0:tk0 + KB],
                                 qT[0:64, tq0 + qc0:tq0 + QT],
                                 tile_position=(0, 0))
                nc.tensor.matmul(ps_s[:, 1, 0:w], kT[64:128, tk0:tk0 + KB],
                                 qT[64:128, tq0 + qc0:tq0 + QT],
                                 tile_position=(64, 0))
                pt = pt_pool.tile([128, 2, QT], dt.bfloat16, tag="pt")
                nc.scalar.activation(pt[:, :, 0:w], ps_s[:, :, 0:w],
                                     AF.Exp, scale=scale)
                if j >= 0:
                    for h in (0, 1):
                        nc.vector.tensor_tensor(
                            pt[:, h, 0:KB], pt[:, h, 0:KB], tri[:], ALU.mult)
                pts[kb] = (pt, qc0, w)

            def av(kb):
                pt, qc0, w = pts.pop(kb)
                st = dict(start=(kb == 0), stop=(kb == nkb - 1))
                nc.tensor.matmul(o65_h0[:, qc0:QT],
                                 vst[:, bb, kb, 0:DH + 1],
                                 pt[:, 0, 0:w], **st)
                nc.tensor.matmul(o65_h1[:, qc0:QT],
                                 vst[:, bb, kb, DH + 1:2 * DH + 2],
                                 pt[:, 1, 0:w], **st)

            scores_exp(0)
            for kb in range(nkb):
                if kb + 1 < nkb:
                    scores_exp(kb + 1)
                pop_fillers(per_slot_u, 0)
                pop_lazy_slot(rate, kb >= 6)
                self_rem_slots[0] -= 1
                av(kb)

            # early-release: one copy per head frees the o65 PSUM banks
            # sooner, so the next tile's first AV (which clears the bank)
            # is not gated on the full normalize chain below.
            tmps = []
            for h, o65 in ((0, o65_h0), (1, o65_h1)):
                tmp = ev_pool.tile([DH + 1, QT], dt.float32, tag="oc",
                                   name="oc", bufs=3)
                nc.vector.tensor_copy(tmp[:], o65[0:DH + 1, :])
                tmps.append(tmp)
            # normalize from the SBUF copies
            bcs = []
            for h in (0, 1):
                tmp = tmps[h]
                row = ev_pool.tile([1, QT], dt.float32, tag="row")
                rec = ev_pool.tile([1, QT], dt.float32, tag="rec")
                bc = ev_pool.tile([64, QT], dt.float32, tag="bc")
                nc.vector.tensor_copy(row[:], tmp[DH:DH + 1, :])
                nc.vector.reciprocal_approx_fast(rec[:], row[:])
                nc.gpsimd.partition_broadcast(bc[:], rec[:])
                bcs.append(bc)
                if not last:
                    nc.vector.tensor_tensor(
                        ctxT[h * DH:(h + 1) * DH, tq0:tq0 + QT],
                        tmp[0:DH, :], bc[:], ALU.mult)
            if last:
                # pipeline the tail: normalize 128-col chunks and run the
                # output projection for each chunk as soon as it lands,
                # with the held-back previous-tile outproj as PE filler.
                ops = outproj_fillers(bb, qt)
                for c in range(QT // 128):
                    sl = slice(c * 128, (c + 1) * 128)
                    pop_fillers(0, 2)
                    for h in (0, 1):
                        nc.vector.tensor_tensor(
                            ctxT[h * DH:(h + 1) * DH,
                                 tq0 + c * 128:tq0 + (c + 1) * 128],
                            tmps[h][0:DH, sl], bcs[h][:, sl], ALU.mult)
                    ops[2 * c]()
                    ops[2 * c + 1]()

        # ---- emission schedule ----
        # attention order A_i = (i%b, i//b); qkv unit Q_i matches; Q_{i+1}
        # is emitted as urgent filler during A_i.
        steps = [(i % b, i // b) for i in range(b * n_qt)]
        self_rem_slots = [sum((qt + 1) * QT // KB for _, qt in steps)]
        xt_load(*steps[0], split=True)
        nc.sync.dma_start(wqkvT[:, :, 256:384], wq_r[:, :, 256:384])
        xt_load(*steps[1])
        nc.sync.dma_start(woutT[:], wout_d[:])
        xt_load(*steps[2])
        for f in qkv_fillers(*steps[0]):
            f()
        for i, (bb, qt) in enumerate(steps):
            if i + 3 < len(steps):
                xt_load(*steps[i + 3])
            if i + 1 < len(steps):
                urgent_q.extend(qkv_fillers(*steps[i + 1]))
            attention(bb, qt)
            if not (bb == b - 1 and qt == n_qt - 1):
                lazy_q.extend(outproj_fillers(bb, qt))
        for f in urgent_q + lazy_q:
            f()

    return nc


def _get_kernel(b, s):
    key = (b, s)
    if key not in _cache:
        from concourse import bacc
        nc = bacc.Bacc()
        _build(nc, b, s)
        nc.finalize()
        _cache[key] = nc
    return _cache[key]


def _prep_inputs(x, Wqkv, Wout):
    """Host-side shard + transpose + bf16 cast. Returns list of in_maps."""
    b, s, d = x.shape
    xT = np.ascontiguousarray(
        x.reshape(b * s, d).astype(ml_dtypes.bfloat16).T)  # (d, b*s)
    n_dblk = d // 128
    in_maps = []
    for i in range(N_CORES):
        r0 = i * 128
        wq = Wqkv[r0:r0 + 128]            # (128, d)
        wk = Wqkv[d + r0:d + r0 + 128]
        wv = Wqkv[2 * d + r0:2 * d + r0 + 128]
        wT = np.concatenate([wq.T, wk.T, wv.T], axis=1)  # (d, 384)
        wT = wT.reshape(n_dblk, 128, 3 * 128).astype(ml_dtypes.bfloat16)
        woT = Wout[:, r0:r0 + 128].T.astype(ml_dtypes.bfloat16)
        woT = np.ascontiguousarray(woT)
        in_maps.append({"xT": xT, "wqkvT": wT, "woutT": woT})
    return in_maps


_runner_cache = {}


def _make_runner(nc, n_cores):
    """Like bass2jax.run_bass_via_pjrt but with the jitted executable built
    once and cached, and output zero-buffers created on-device instead of
    being uploaded every call."""
    import jax
    import jax.numpy as jnp
    from jax.sharding import Mesh, PartitionSpec
    from jax.experimental.shard_map import shard_map
    import concourse.mybir as mybir
    from concourse import bass2jax

    bass2jax.install_neuronx_cc_hook()
    partition_name = (nc.partition_id_tensor.name
                      if nc.partition_id_tensor else None)
    in_names, out_names, out_avals = [], [], []
    for alloc in nc.m.functions[0].allocations:
        if not isinstance(alloc, mybir.MemoryLocationSet):
            continue
        name = alloc.memorylocations[0].name
        if alloc.kind == "ExternalInput":
            if name != partition_name:
                in_names.append(name)
        elif alloc.kind == "ExternalOutput":
            out_names.append(name)
            out_avals.append(jax.core.ShapedArray(
                tuple(alloc.tensor_shape), mybir.dt.np(alloc.dtype)))
    n_params = len(in_names)
    n_outs = len(out_names)
    bind_names = list(in_names) + list(out_names)
    if partition_name is not None:
        bind_names.append(partition_name)

    def _body(*args):
        operands = list(args)
        if partition_name is not None:
            operands.append(bass2jax.partition_id_tensor())
        outs = bass2jax._bass_exec_p.bind(
            *operands,
            out_avals=tuple(out_avals),
            in_names=tuple(bind_names),
            out_names=tuple(out_names),
            lowering_input_output_aliases=(),
            sim_require_finite=True,
            sim_require_nnan=True,
            nc=nc,
        )
        return tuple(outs)

    devices = jax.devices()[:n_cores]
    mesh = Mesh(np.array(devices), ("core",))
    sharded = jax.jit(
        shard_map(
            _body, mesh=mesh,
            in_specs=(PartitionSpec("core"),) * (n_params + n_outs),
            out_specs=(PartitionSpec("core"),) * n_outs,
            check_rep=False),
        donate_argnums=tuple(range(n_params, n_params + n_outs)),
        keep_unused=True)

    def run(in_maps):
        concat_in = [
            np.concatenate([np.asarray(m[name]) for m in in_maps], axis=0)
            for name in in_names]
        concat_zeros = [
            np.zeros((n_cores * a.shape[0], *a.shape[1:]), a.dtype)
            for a in out_avals]
        out_arrs = sharded(*concat_in, *concat_zeros)
        return [
            {name: np.asarray(out_arrs[i]).reshape(
                n_cores, *out_avals[i].shape)[c]
             for i, name in enumerate(out_names)}
            for c in range(n_cores)]

    return run


def kernel(x, Wqkv, Wout, _trace=False):
    b, s, d = x.shape
    nc = _get_kernel(b, s)
    in_maps = _prep_inputs(np.asarray(x), np.asarray(Wqkv), np.asarray(Wout))
    if _trace:
        from concourse.bass_utils import run_bass_kernel_spmd
        res = run_bass_kernel_spmd(nc, in_maps,
                                   core_ids=list(range(N_CORES)), trace=True)
        results = res.results
        kernel.last_results = res
    else:
        key = id(nc)
        if key not in _runner_cache:
            _runner_cache[key] = _make_runner(nc, N_CORES)
        results = _runner_cache[key](in_maps)
    acc = results[0]["partial_out"].astype(np.float32)
    for i in range(1, N_CORES):
        acc = acc + results[i]["partial_out"]
    return acc.reshape(b, s, d)


# revision 18
# speedup vs baseline: 1.0038x; 1.0038x over previous
"""Causal multi-head self-attention on 8 Trainium2 NeuronCores.

Sharding: head-parallel. Each of the 8 cores owns 2 of the 16 heads:
it computes Q/K/V for its heads (full sequence), runs causal flash
attention for them entirely on-chip, applies its slice of the output
projection, and writes a full-shape partial output. The host sums the
8 partials.

v2 schedule: the kernel is jointly PE- and ACT(exp)-bound, so the
emission order interleaves at kb-block granularity: QKV-projection and
output-projection matmuls are queued as "fillers" and dropped one or
two at a time between the score/AV matmuls of the attention inner
loop. The PE never idles (stays at max p-state) while the Scalar
engine streams exp calls; output projection runs inline per q-tile so
its DMA overlaps the whole kernel instead of forming a tail.

Layout:
  - x is cast to bf16 on host and staged transposed; one DMA per
    (batch, 512-token) tile loads all 8 d-blocks.
  - Q^T, K^T are (128 = [h0|h1] x 64) x t, the exact lhsT/rhs layout
    the transposed score matmuls need; score pairs dual-issue on the
    PE via row-disjoint tile_position quadrants.
  - exp runs on ScalarE straight out of PSUM (scale=1/8 fused), a
    single call per k-block covering both heads (3D AP on diagonals).
  - V is stored per (batch, kblock) as 129 columns [v_h0 | ones |
    v_h1]; the shared ones column makes both heads' AV matmuls emit
    the softmax denominator as an extra output row for free.
  - Causal masking: diagonal blocks are narrowed to the valid q range
    and the 128-column boundary gets a precomputed 0/1 triangle
    multiply after exp.
  - Normalization at AV eviction: reciprocal row broadcast over
    partitions (GpSimd) then one fused multiply PSUM->SBUF into ctx^T.
  - Output projection consumes ctx^T blocks as stationary operands so
    results land (t x e); one DMA per (batch, q-tile) writes them out.
"""

import numpy as np
import sys

for _p in ("/opt/trn_rl_repo", "/root/.axon_site/_ro/trn_rl_repo"):
    if _p not in sys.path:
        sys.path.append(_p)

import ml_dtypes

B = 2
S = 4096
D = 1024
H = 16
DH = 64
N_CORES = 8
HEADS_PER_CORE = H // N_CORES  # 2

_cache = {}


def _build(nc, b, s):
    import concourse.bass as bass
    import concourse.mybir as mybir
    from concourse.tile import TileContext
    from contextlib import ExitStack

    dt = mybir.dt
    AF = mybir.ActivationFunctionType
    ALU = mybir.AluOpType

    t_total = b * s          # 8192
    TT = 512                 # t tile (QKV free dim)
    n_dblk = D // 128        # 8
    QT = 512                 # q tile
    n_qt = s // QT           # per batch (8)
    KB = 128                 # k block
    n_kblk = s // KB         # 32
    scale = 1.0 / np.sqrt(DH)

    x_d = nc.dram_tensor("xT", [D, t_total], dt.bfloat16, kind="ExternalInput")
    wqkv_d = nc.dram_tensor("wqkvT", [n_dblk, 128, 3 * 128], dt.bfloat16,
                            kind="ExternalInput")
    wout_d = nc.dram_tensor("woutT", [128, D], dt.bfloat16, kind="ExternalInput")
    out_d = nc.dram_tensor("partial_out", [t_total, D], dt.bfloat16,
                           kind="ExternalOutput")

    with TileContext(nc) as tc, ExitStack() as ctx:
        const = ctx.enter_context(tc.tile_pool(name="const", bufs=1))
        wqkvT = const.tile([128, n_dblk, 3 * 128], dt.bfloat16, tag="wqkv")
        woutT = const.tile([128, D], dt.bfloat16, tag="wout")
        qT = const.tile([128, t_total], dt.bfloat16, tag="qT")
        kT = const.tile([128, t_total], dt.bfloat16, tag="kT")
        # V: per (batch, kblock) 130 cols [v_h0 | ones | v_h1 | ones]
        vst = const.tile([128, b, n_kblk, 2 * DH + 2], dt.bfloat16, tag="vst")
        ctxT = const.tile([128, t_total], dt.bfloat16, tag="ctxT")
        tri = const.tile([128, 128], dt.bfloat16, tag="tri")
        ident = const.tile([128, 128], dt.bfloat16, tag="ident")

        # split the weight load so the first QKV matmuls (which need only
        # the q|k columns) start as early as possible; wout is needed last.
        wq_r = wqkv_d.rearrange("k p e -> p k e")
        nc.sync.dma_start(wqkvT[:, :, 0:256], wq_r[:, :, 0:256])

        # ones column of vst, the 0/1 lower-triangle mask (keep k<=q: in
        # (k=partition r, q=col c) space keep c >= r), and the identity
        # for the PE transpose of V.
        nc.vector.memset(vst[:, :, :, DH], 1.0)
        nc.vector.memset(vst[:, :, :, 2 * DH + 1], 1.0)
        nc.gpsimd.memset(tri[:], 1.0)
        nc.gpsimd.affine_select(
            tri[:], tri[:], pattern=[[1, 128]], compare_op=ALU.is_ge,
            fill=0.0, base=0, channel_multiplier=-1,
        )
        nc.gpsimd.affine_select(
            ident[:], tri[:], pattern=[[1, 128]], compare_op=ALU.is_equal,
            fill=0.0, base=0, channel_multiplier=-1,
        )

        # SBUF pools
        xt_pool = ctx.enter_context(tc.tile_pool(name="xt", bufs=4))
        pt_pool = ctx.enter_context(tc.tile_pool(name="pt", bufs=6))
        vt_pool = ctx.enter_context(tc.tile_pool(name="vt", bufs=2))
        ev_pool = ctx.enter_context(tc.tile_pool(name="ev", bufs=4))
        out_sb_pool = ctx.enter_context(tc.tile_pool(name="out_sb", bufs=3))
        # PSUM: 8 banks = scores 2x2 + o65 2 + skip(qkv/transpose/outproj) 2
        sc_ps = ctx.enter_context(tc.tile_pool(name="sc_ps", bufs=2, space="PSUM"))
        o65_ps = ctx.enter_context(tc.tile_pool(name="o65_ps", bufs=1, space="PSUM"))
        skip_ps = ctx.enter_context(tc.tile_pool(name="skip_ps", bufs=2,
                                                 space="PSUM"))

        xt_tiles = {}

        def xt_load(bb, tt, split=False):
            """One DMA: all 8 d-blocks of a (bb, tt) token tile. split=True
            issues two half DMAs so the first QKV matmuls start sooner."""
            xt = xt_pool.tile([128, n_dblk, TT], dt.bfloat16, tag="xt")
            t0 = bb * s + tt * TT
            xsrc = x_d.rearrange("(k p) t -> p k t", p=128)[:, :, t0:t0 + TT]
            if split:
                nc.sync.dma_start(xt[:, 0:4, :], xsrc[:, 0:4, :])
                nc.sync.dma_start(xt[:, 4:8, :], xsrc[:, 4:8, :])
            else:
                nc.sync.dma_start(xt[:], xsrc)
            xt_tiles[(bb, tt)] = xt

        def qkv_fillers(bb, tt):
            """Emit QKV projection for (bb, tt) as a list of PE closures.

            q and k accumulate into the two skip-pool slots; v reuses
            q's slot after eviction, then 4 PE transposes scatter V into
            vst via two DVE copies each.
            """
            t0 = bb * s + tt * TT
            xt = xt_tiles.pop((bb, tt))
            state = {}

            def mk_qk(dd):
                def emit():
                    if dd == 0:
                        state["ps_q"] = skip_ps.tile([128, TT], dt.float32,
                                                     name="ps_q", tag="skip")
                        state["ps_k"] = skip_ps.tile([128, TT], dt.float32,
                                                     name="ps_k", tag="skip")
                    st = dict(start=(dd == 0), stop=(dd == n_dblk - 1))
                    nc.tensor.matmul(state["ps_q"][:], wqkvT[:, dd, 0:128],
                                     xt[:, dd, :], **st)
                    nc.tensor.matmul(state["ps_k"][:], wqkvT[:, dd, 128:256],
                                     xt[:, dd, :], **st)
                    if dd == n_dblk - 1:
                        nc.vector.tensor_copy(qT[:, t0:t0 + TT], state["ps_q"][:])
                        nc.vector.tensor_copy(kT[:, t0:t0 + TT], state["ps_k"][:])
                return emit

            def mk_v(dd):
                def emit():
                    if dd == 0:
                        state["ps_v"] = skip_ps.tile([128, TT], dt.float32,
                                                     name="ps_v", tag="skip")
                    st = dict(start=(dd == 0), stop=(dd == n_dblk - 1))
                    nc.tensor.matmul(state["ps_v"][:], wqkvT[:, dd, 256:384],
                                     xt[:, dd, :], **st)
                    if dd == n_dblk - 1:
                        vt = vt_pool.tile([128, TT], dt.bfloat16, tag="vt")
                        nc.vector.tensor_copy(vt[:], state["ps_v"][:])
                        state["vt"] = vt
                return emit

            def mk_tr(j):
                def emit():
                    ps_tv = skip_ps.tile([128, 128], dt.bfloat16, name="ps_tv",
                                         tag="skip")
                    nc.tensor.transpose(ps_tv[:],
                                        state["vt"][:, j * 128:(j + 1) * 128],
                                        ident[:])
                    kb = (tt * TT) // KB + j
                    nc.vector.tensor_copy(vst[:, bb, kb, 0:DH], ps_tv[:, 0:DH])
                    nc.vector.tensor_copy(vst[:, bb, kb, DH + 1:2 * DH + 1],
                                          ps_tv[:, DH:2 * DH])
                    # (col DH and col 2*DH+1 hold the ones columns)
                return emit

            return ([mk_qk(dd) for dd in range(n_dblk)]
                    + [mk_v(dd) for dd in range(n_dblk)]
                    + [mk_tr(j) for j in range(TT // 128)])

        def outproj_fillers(bb, qt):
            """Output projection for (bb, qt): 8 matmul closures + 1 DMA."""
            tq0 = bb * s + qt * QT
            state = {}

            def mk(i):
                tb, e = divmod(i, 2)

                def emit():
                    if i == 0:
                        state["ob"] = out_sb_pool.tile(
                            [128, QT // 128, D], dt.bfloat16, name="ob",
                            tag="ob")
                    ps = skip_ps.tile([128, 512], dt.float32, name="ps_op",
                                      tag="skip")
                    t0 = tq0 + tb * 128
                    nc.tensor.matmul(ps[:], ctxT[:, t0:t0 + 128],
                                     woutT[:, e * 512:(e + 1) * 512])
                    nc.vector.tensor_copy(
                        state["ob"][:, tb, e * 512:(e + 1) * 512], ps[:])
                    if i == 7:
                        nc.sync.dma_start(
                            out_d.rearrange("(u j p) e -> p u j e", p=128,
                                            j=QT // 128)[:, bb * n_qt + qt],
                            state["ob"][:])
                return emit

            return [mk(i) for i in range(8)]

        urgent_q = []
        lazy_q = []

        def pop_fillers(n_urgent, n_lazy):
            for _ in range(min(n_urgent, len(urgent_q))):
                urgent_q.pop(0)()
            for _ in range(min(n_lazy, len(lazy_q))):
                lazy_q.pop(0)()

        def attention(bb, qt):
            """One q-tile of causal attention for both heads of batch bb.

            Emits scores(kb+1) before AV(kb) so the PE is never
            head-of-line blocked on exp(kb); fillers are popped between
            iterations to absorb the remaining ACT/PE rate mismatch.
            """
            tq0 = bb * s + qt * QT
            o65_h0 = o65_ps.tile([DH + 1, QT], dt.float32, tag="o65h0")
            o65_h1 = o65_ps.tile([DH + 1, QT], dt.float32, tag="o65h1")
            nkb = (qt + 1) * QT // KB
            # urgent fillers (next tile's QKV) must drain within this
            # attention; lazy fillers (outproj) pace uniformly over the
            # remaining kb slots of the whole kernel so the late q-tiles
            # (which have no QKV work left) still get PE filler.
            per_slot_u = max(1, -(-len(urgent_q) // nkb))
            rem_slots = self_rem_slots[0]
            last = (bb == b - 1 and qt == n_qt - 1)
            # uniform credit pacing: the outproj backlog drains evenly over
            # ALL remaining kb slots of the kernel, so the late q-tiles
            # (which have no QKV filler) keep PE work; the last tile holds
            # back a reserve for its normalize/outproj tail.
            reserve = 8 if last else 0
            rate = max(len(lazy_q) - reserve, 0) / max(rem_slots, 1)
            credit = [0.0]

            pts = {}

            def scores_exp(kb):
                tk0 = bb * s + kb * KB
                j = kb - qt * (QT // KB)  # >= 0 on the diagonal
                qc0 = max(j, 0) * KB      # first valid local q column
                w = QT - qc0
                ps_s = sc_ps.tile([128, 2, QT], dt.float32, tag="ps_s")
                nc.tensor.matmul(ps_s[:, 0, 0:w], kT[0:64, tk0:tk0 + KB],
                                 qT[0:64, tq0 + qc0:tq0 + QT],
                                 tile_position=(0, 0))
                nc.tensor.matmul(ps_s[:, 1, 0:w], kT[64:128, tk0:tk0 + KB],
                                 qT[64:128, tq0 + qc0:tq0 + QT],
                                 tile_position=(64, 0))
                pt = pt_pool.tile([128, 2, QT], dt.bfloat16, tag="pt")
                nc.scalar.activation(pt[:, :, 0:w], ps_s[:, :, 0:w],
                                     AF.Exp, scale=scale)
                if j >= 0:
                    for h in (0, 1):
                        nc.vector.tensor_tensor(
                            pt[:, h, 0:KB], pt[:, h, 0:KB], tri[:], ALU.mult)
                pts[kb] = (pt, qc0, w)

            def av(kb):
                pt, qc0, w = pts.pop(kb)
                st = dict(start=(kb == 0), stop=(kb == nkb - 1))
                nc.tensor.matmul(o65_h0[:, qc0:QT],
                                 vst[:, bb, kb, 0:DH + 1],
                                 pt[:, 0, 0:w], **st)
                nc.tensor.matmul(o65_h1[:, qc0:QT],
                                 vst[:, bb, kb, DH + 1:2 * DH + 2],
                                 pt[:, 1, 0:w], **st)

            scores_exp(0)
            for kb in range(nkb):
                if kb + 1 < nkb:
                    scores_exp(kb + 1)
                credit[0] += rate
                n_l = int(credit[0])
                credit[0] -= n_l
                pop_fillers(per_slot_u, n_l)
                self_rem_slots[0] -= 1
                av(kb)

            # early-release: one copy per head frees the o65 PSUM banks
            # sooner, so the next tile's first AV (which clears the bank)
            # is not gated on the full normalize chain below.
            tmps = []
            for h, o65 in ((0, o65_h0), (1, o65_h1)):
                tmp = ev_pool.tile([DH + 1, QT], dt.float32, tag="oc",
                                   name="oc", bufs=3)
                nc.vector.tensor_copy(tmp[:], o65[0:DH + 1, :])
                tmps.append(tmp)
            # normalize from the SBUF copies
            bcs = []
            for h in (0, 1):
                tmp = tmps[h]
                row = ev_pool.tile([1, QT], dt.float32, tag="row")
                rec = ev_pool.tile([1, QT], dt.float32, tag="rec")
                bc = ev_pool.tile([64, QT], dt.float32, tag="bc")
                nc.vector.tensor_copy(row[:], tmp[DH:DH + 1, :])
                nc.vector.reciprocal_approx_fast(rec[:], row[:])
                nc.gpsimd.partition_broadcast(bc[:], rec[:])
                bcs.append(bc)
                if not last:
                    nc.vector.tensor_tensor(
                        ctxT[h * DH:(h + 1) * DH, tq0:tq0 + QT],
                        tmp[0:DH, :], bc[:], ALU.mult)
            if last:
                # pipeline the tail: normalize 128-col chunks and run the
                # output projection for each chunk as soon as it lands,
                # with the held-back previous-tile outproj as PE filler.
                ops = outproj_fillers(bb, qt)
                for c in range(QT // 128):
                    sl = slice(c * 128, (c + 1) * 128)
                    pop_fillers(0, 2)
                    for h in (0, 1):
                        nc.vector.tensor_tensor(
                            ctxT[h * DH:(h + 1) * DH,
                                 tq0 + c * 128:tq0 + (c + 1) * 128],
                            tmps[h][0:DH, sl], bcs[h][:, sl], ALU.mult)
                    ops[2 * c]()
                    ops[2 * c + 1]()

        # ---- emission schedule ----
        # attention order A_i = (i%b, i//b); qkv unit Q_i matches; Q_{i+1}
        # is emitted as urgent filler during A_i.
        steps = [(i % b, i // b) for i in range(b * n_qt)]
        self_rem_slots = [sum((qt + 1) * QT // KB for _, qt in steps)]
        xt_load(*steps[0], split=True)
        nc.sync.dma_start(wqkvT[:, :, 256:384], wq_r[:, :, 256:384])
        xt_load(*steps[1])
        nc.sync.dma_start(woutT[:], wout_d[:])
        xt_load(*steps[2])
        for f in qkv_fillers(*steps[0]):
            f()
        for i, (bb, qt) in enumerate(steps):
            if i + 3 < len(steps):
                xt_load(*steps[i + 3])
            if i + 1 < len(steps):
                urgent_q.extend(qkv_fillers(*steps[i + 1]))
            attention(bb, qt)
            if not (bb == b - 1 and qt == n_qt - 1):
                lazy_q.extend(outproj_fillers(bb, qt))
        for f in urgent_q + lazy_q:
            f()

    return nc


def _get_kernel(b, s):
    key = (b, s)
    if key not in _cache:
        from concourse import bacc
        nc = bacc.Bacc()
        _build(nc, b, s)
        nc.finalize()
        _cache[key] = nc
    return _cache[key]


def _prep_inputs(x, Wqkv, Wout):
    """Host-side shard + transpose + bf16 cast. Returns list of in_maps."""
    b, s, d = x.shape
    xT = np.ascontiguousarray(
        x.reshape(b * s, d).astype(ml_dtypes.bfloat16).T)  # (d, b*s)
    n_dblk = d // 128
    in_maps = []
    for i in range(N_CORES):
        r0 = i * 128
        wq = Wqkv[r0:r0 + 128]            # (128, d)
        wk = Wqkv[d + r0:d + r0 + 128]
        wv = Wqkv[2 * d + r0:2 * d + r0 + 128]
        wT = np.concatenate([wq.T, wk.T, wv.T], axis=1)  # (d, 384)
        wT = wT.reshape(n_dblk, 128, 3 * 128).astype(ml_dtypes.bfloat16)
        woT = Wout[:, r0:r0 + 128].T.astype(ml_dtypes.bfloat16)
        woT = np.ascontiguousarray(woT)
        in_maps.append({"xT": xT, "wqkvT": wT, "woutT": woT})
    return in_maps


_runner_cache = {}


def _make_runner(nc, n_cores):
    """Like bass2jax.run_bass_via_pjrt but with the jitted executable built
    once and cached, and output zero-buffers created on-device instead of
    being uploaded every call."""
    import jax
    import jax.numpy as jnp
    from jax.sharding import Mesh, PartitionSpec
    from jax.experimental.shard_map import shard_map
    import concourse.mybir as mybir
    from concourse import bass2jax

    bass2jax.install_neuronx_cc_hook()
    partition_name = (nc.partition_id_tensor.name
                      if nc.partition_id_tensor else None)
    in_names, out_names, out_avals = [], [], []
    for alloc in nc.m.functions[0].allocations:
        if not isinstance(alloc, mybir.MemoryLocationSet):
            continue
        name = alloc.memorylocations[0].name
        if alloc.kind == "ExternalInput":
            if name != partition_name:
                in_names.append(name)
        elif alloc.kind == "ExternalOutput":
            out_names.append(name)
            out_avals.append(jax.core.ShapedArray(
                tuple(alloc.tensor_shape), mybir.dt.np(alloc.dtype)))
    n_params = len(in_names)
    n_outs = len(out_names)
    bind_names = list(in_names) + list(out_names)
    if partition_name is not None:
        bind_names.append(partition_name)

    def _body(*args):
        operands = list(args)
        if partition_name is not None:
            operands.append(bass2jax.partition_id_tensor())
        outs = bass2jax._bass_exec_p.bind(
            *operands,
            out_avals=tuple(out_avals),
            in_names=tuple(bind_names),
            out_names=tuple(out_names),
            lowering_input_output_aliases=(),
            sim_require_finite=True,
            sim_require_nnan=True,
            nc=nc,
        )
        return tuple(outs)

    devices = jax.devices()[:n_cores]
    mesh = Mesh(np.array(devices), ("core",))
    sharded = jax.jit(
        shard_map(
            _body, mesh=mesh,
            in_specs=(PartitionSpec("core"),) * (n_params + n_outs),
            out_specs=(PartitionSpec("core"),) * n_outs,
            check_rep=False),
        donate_argnums=tuple(range(n_params, n_params + n_outs)),
        keep_unused=True)

    def run(in_maps):
        concat_in = [
            np.concatenate([np.asarray(m[name]) for m in in_maps], axis=0)
            for name in in_names]
        concat_zeros = [
            np.zeros((n_cores * a.shape[0], *a.shape[1:]), a.dtype)
            for a in out_avals]
        out_arrs = sharded(*concat_in, *concat_zeros)
        return [
            {name: np.asarray(out_arrs[i]).reshape(
                n_cores, *out_avals[i].shape)[c]
             for i, name in enumerate(out_names)}
            for c in range(n_cores)]

    return run


def kernel(x, Wqkv, Wout, _trace=False):
    b, s, d = x.shape
    nc = _get_kernel(b, s)
    in_maps = _prep_inputs(np.asarray(x), np.asarray(Wqkv), np.asarray(Wout))
    if _trace:
        from concourse.bass_utils import run_bass_kernel_spmd
        res = run_bass_kernel_spmd(nc, in_maps,
                                   core_ids=list(range(N_CORES)), trace=True)
        results = res.results
        kernel.last_results = res
    else:
        key = id(nc)
        if key not in _runner_cache:
            _runner_cache[key] = _make_runner(nc, N_CORES)
        results = _runner_cache[key](in_maps)
    acc = results[0]["partial_out"].astype(np.float32)
    for i in range(1, N_CORES):
        acc = acc + results[i]["partial_out"]
    return acc.reshape(b, s, d)
